# revision 78
# baseline (speedup 1.0000x reference)
"""Depth-modulated 3x3 conv (DepthConv) for Trainium2, 8-way batch-parallel.

out(b,o,h,w) = sum_{c,i,j} W[o,c,i,j] * x[b,c,h+i-1,w+j-1]
               * exp(-8.3*|d[b,h,w] - d[b,h+i-1,w+j-1]|)

Strategy (per core = one batch item):
  - Pixels are matmul OUTPUT partitions: 32 tiles of 128 px (2 rows).
  - For each row-shift i in {0,1,2}: stationary lhsT = x[cin_chunk, 128 px
    shifted by (i-1)*64] (bf16), moving rhs = W[cin_chunk, (j,o)=192] (bf16),
    4 cin chunks accumulate in PSUM -> y3[p, i, (j,o)].
  - Gates exp(-8.3|dd|) are precomputed AND cout-replicated on the HOST.
    Tiles 0-7 get 32-wide gates (two half-width multiplies; less input
    during the delivery-bound lead-in), tiles 8-31 get full 64-wide gates
    (one contiguous 2x-bf16 multiply, ~460ns, keeps DVE under the PE
    period). No on-device replication build.
  - i-sum: add1 on DVE and add2 on the Pool engine, both EMITTED one tile
    late. Every same-engine RAW is then a full period old (no completion
    wait) and the only cross-engine wait (Pool on DVE's monotonic counter)
    never lags; Pool is half-idle so the ~0.7us sem latency pipelines
    away. (Putting any Pool-produced value on DVE's queue head loses
    ~0.5us/tile to sem round-trips -- measured, twice.)
  - Output partials are fp16 (halves output DMA; |partials| <= ~4 so fp16
    rounding is ~1e-3 per stream). Host combines the three j-aligned
    streams: out[q] = P0[q-1] + P1[q] + P2[q+1].
  - All DMAs ride the sync queue in strict consumption order (w, x column
    blocks of >=640 cols so per-partition descriptor runs stay >=1.25KB
    for full DMA rate, gate chunks interleaved at their consumption
    points, out-DMA triggers emitted inline right after each half-group's
    last add2). Deep tile rings (work bufs=8, all 8 half-group buffers
    live) keep transient lags from locking into sem-latency limit cycles.
  - ~65 dummy matmuls on a memset scratch tile run during the DMA lead-in
    to pull the PE clock gate (HAM) toward 2.4GHz before real work.
"""
import os
import sys
sys.path.insert(0, '/opt/trn_rl_repo')

import numpy as np
import ml_dtypes

import concourse.bass as bass
import concourse.tile as tile
from concourse import bacc, mybir
from concourse.bass_utils import run_bass_kernel_spmd

F32 = mybir.dt.float32
F16 = mybir.dt.float16
BF16 = mybir.dt.bfloat16

B, CIN, H, W = 8, 512, 64, 64
COUT, K = 64, 3
ALPHA = 8.3
NPX = H * W            # 4096
NT = NPX // 128        # 32 pixel tiles
KC = CIN // 128        # 4 cin chunks
XCOLS = NPX + 128      # 64 guard + 4096 + 64 guard
HG = 4                 # tiles per half-group (output DMA granularity)
NH = NT // HG          # 8 half-groups
GW = 32                # host-replicated gate width (cout halves share it)
# consumption-ordered x column blocks; in DRAM each block is packed
# contiguously per partition ([KC, bw] in one ~5KB run) so every DMA
# descriptor is one fat run instead of four ~1.25KB strided runs
XBLOCKS = [(256 * n, 256 * (n + 1)) for n in range(16)] + [(4096, 4224)]
XOFF = [0]
for _c0, _c1 in XBLOCKS:
    XOFF.append(XOFF[-1] + KC * (_c1 - _c0))

_cache = {}


def build_nc():
    nc = bacc.Bacc("TRN2", target_bir_lowering=False, debug=False, num_devices=B)
    x_d = nc.dram_tensor("x", [128, XOFF[-1]], BF16, kind="ExternalInput").ap()
    w_d = nc.dram_tensor("w", [128, KC, 3, 192], BF16, kind="ExternalInput").ap()
    # tiles 0-7: 32-wide gates (less input during the delivery-bound lead-in,
    # gated as two half-mults); tiles 8-31: full 64-wide gates (single
    # contiguous 2x-mode multiply keeps DVE far under the PE period)
    g_d = nc.dram_tensor("g", [128, 8, 9, GW], BF16, kind="ExternalInput").ap()
    g64_d = nc.dram_tensor("g64", [128, NT - 8, 9, COUT], BF16,
                           kind="ExternalInput").ap()
    # three j-aligned partial streams; host applies the +-1 pixel shift + sum
    out_d = nc.dram_tensor("out", [NPX, 3, COUT], F16, kind="ExternalOutput").ap()

    with tile.TileContext(nc) as tc:
        with tc.tile_pool(name="const", bufs=1) as cpool, \
             tc.tile_pool(name="work", bufs=8) as wpool, \
             tc.tile_pool(name="phg", bufs=8) as hgpool, \
             tc.tile_pool(name="psum", bufs=4, space="PSUM") as ppool:

            # PE pre-warm: dummy matmuls on a memset scratch tile pull the
            # HAM clock gate toward 2.4GHz while the real inputs stream in.
            scratch = cpool.tile([128, 128], BF16)
            nc.gpsimd.memset(scratch[:], 0.0)
            ps_dummy = ppool.tile([128, 3, 256], F32, tag="ps",
                                  name="ps_dummy")
            for _ in range(110):
                nc.tensor.matmul(ps_dummy[:, 0, 0:64], scratch[:],
                                 scratch[:, 0:64], start=True, stop=True)

            w_sb = cpool.tile([128, KC, 3, 192], BF16)
            x_sb = cpool.tile([128, KC, XCOLS], BF16)
            g_sb = cpool.tile([128, 8, 9, GW], BF16)
            g64_sb = cpool.tile([128, NT - 8, 9, COUT], BF16)

            # x column blocks in consumption order, each ONE dma_start
            # covering all 4 cin chunks from a block-contiguous DRAM run
            def x_block(n):
                c0, c1 = XBLOCKS[n]
                src = x_d[:, XOFF[n]:XOFF[n + 1]].rearrange(
                    "p (k c) -> p k c", k=KC)
                nc.sync.dma_start(x_sb[:, :, c0:c1], src)

            def g_chunk(n):
                nc.sync.dma_start(g_sb[:, 4 * n:4 * n + 4],
                                  g_d[:, 4 * n:4 * n + 4])

            def g64_chunk(n):
                # 2-tile chunks (2.3KB runs): the 4-tile version co-binds
                # the lead-in at the same level as the t=0 chunk
                nc.sync.dma_start(g64_sb[:, 2 * n:2 * n + 2],
                                  g64_d[:, 2 * n:2 * n + 2])

            # out-DMA triggers are emitted inside the trigger stream where
            # their data lands ~2us before the trigger's enqueue turn
            phg_tiles = {}
            out_emitted = set()

            def get_phg(hg_i):
                if hg_i not in phg_tiles:
                    p_hg = hgpool.tile([128, HG, 3, COUT], F16, tag="phg",
                                       name=f"p_hg{hg_i}")
                    phg_tiles[hg_i] = p_hg
                return phg_tiles[hg_i]

            def emit_out(hg_i, half):
                # 2-tile out chunks: fire earlier, smaller bursts on the
                # 8-core-contended HBM, and a 0.098MB final transfer
                out_emitted.add((hg_i, half))
                r0 = 512 * hg_i + 256 * half
                dst = out_d[r0:r0 + 256].rearrange(
                    "(t p) j o -> p t j o", p=128)
                nc.sync.dma_start(dst, get_phg(hg_i)[:, 2 * half:2 * half + 2])

            # ---- steady pipeline emission -------------------------------
            # PE -> ACT cast -> DVE (gate-mult halves + add1, all of DVE's
            # deps are on-pace ACT work) -> Pool add2 (its only cross-engine
            # wait is DVE's monotonic counter, which never lags; Pool is
            # half-idle so the ~0.7us sem latency pipelines away) -> out DMA
            # i-sum, deferred one tile: add1 on DVE (same-engine tmp read is
            # then a full period old -> no completion-wait), add2 on Pool
            # (its DVE-count wait never lags; Pool is half-idle so the
            # ~0.7us cross-engine sem latency pipelines away). Any variant
            # that puts a Pool-produced value on DVE's queue (even 4 tiles
            # staggered) re-creates a ~1.5us/tile sem limit cycle in the
            # drain -- measured three times.
            def emit_sums(pt, ptmp):
                hg_i, th_i = pt // HG, pt % HG
                s1 = wpool.tile([128, 3, COUT], BF16, tag="s1",
                                name=f"s1_{pt}")
                nc.vector.tensor_tensor(s1[:], ptmp[:, 0], ptmp[:, 1],
                                        op=mybir.AluOpType.add)
                nc.gpsimd.tensor_tensor(get_phg(hg_i)[:, th_i, :, :],
                                        s1[:], ptmp[:, 2],
                                        op=mybir.AluOpType.add)
                if th_i % 2 == 1:
                    emit_out(hg_i, th_i // 2)

            # all input triggers up-front in strict consumption order (the
            # sync queue runs them serially; out triggers join the stream
            # later, emitted inline right after each half-group completes)
            def emit_triggers(t):
                if t == 0:
                    nc.sync.dma_start(w_sb[:], w_d[:])
                    # per-2-tile x chunks so early bytes bind at later
                    # tiles (delivery(cum bytes) + (32-t)*996 is the lead-in
                    # bound); gate chunks ride later than their tiles since
                    # a late gate only delays DVE (slack + deep rings absorb
                    # it) while a late x chunk stalls the PE directly
                    x_block(0)
                    x_block(1)
                    x_block(2)
                    g_chunk(0)
                    x_block(3)
                    x_block(4)
                    g_chunk(1)
                    x_block(5)
                    x_block(6)
                    g64_chunk(0)
                    x_block(7)
                    g64_chunk(1)
                    x_block(8)
                    g64_chunk(2)
                    x_block(9)
                    g64_chunk(3)
                    x_block(10)
                    g64_chunk(4)
                    x_block(11)
                    g64_chunk(5)
                    x_block(12)
                    g64_chunk(6)
                    x_block(13)
                    g64_chunk(7)
                    x_block(14)
                    g64_chunk(8)
                    x_block(15)
                    g64_chunk(9)
                    x_block(16)
                    g64_chunk(10)
                    g64_chunk(11)

            pend = None
            for t in range(NT):
                get_phg(t // HG)
                emit_triggers(t)
                ps = ppool.tile([128, 3, 256], F32, tag="ps")
                for i in range(3):
                    base = 64 + t * 128 + (i - 1) * 64
                    for k in range(KC):
                        nc.tensor.matmul(
                            ps[:, i, 0:192],
                            x_sb[:, k, base:base + 128],
                            w_sb[:, k, i, :],
                            start=(k == 0), stop=(k == KC - 1),
                        )
                # ACT casts PSUM -> bf16 SBUF; DVE gating runs 2x bf16
                y_bf = wpool.tile([128, 3, 192], BF16, tag="ybf")
                nc.scalar.copy(y_bf[:], ps[:, 0:3, 0:192])
                tmp = wpool.tile([128, 3, 3, COUT], BF16, tag="tmp")
                y4 = y_bf[:].rearrange("p i (j o) -> p i j o", j=3)
                if t < 8:
                    g_t = g_sb[:, t, :, :].rearrange("p (i j) o -> p i j o",
                                                     i=3)
                    nc.vector.tensor_tensor(tmp[:, :, :, 0:GW],
                                            y4[:, :, :, 0:GW],
                                            g_t, op=mybir.AluOpType.mult)
                    nc.vector.tensor_tensor(tmp[:, :, :, GW:COUT],
                                            y4[:, :, :, GW:COUT],
                                            g_t, op=mybir.AluOpType.mult)
                else:
                    g_t = g64_sb[:, t - 8, :, :].rearrange(
                        "p (i j) o -> p i j o", i=3)
                    nc.vector.tensor_tensor(tmp[:], y4,
                                            g_t, op=mybir.AluOpType.mult)
                if pend is not None:
                    emit_sums(*pend)
                pend = (t, tmp)
            emit_sums(*pend)
            for hg_i in range(NH):
                for half in range(2):
                    if (hg_i, half) not in out_emitted:
                        emit_out(hg_i, half)

    nc.compile()
    return nc


def prep_inputs(input, depth, weight):
    """Host-side relayout: returns per-core in_maps."""
    # x: (B, 512, 64, 64) -> [128, KC, XCOLS] bf16 with zero guards,
    # then repacked so each XBLOCKS column block is contiguous in DRAM
    xr = input.reshape(B, KC, 128, NPX).transpose(0, 2, 1, 3)  # [B,128,KC,NPX]
    x_all = np.zeros((B, 128, KC, XCOLS), dtype=ml_dtypes.bfloat16)
    x_all[:, :, :, 64:64 + NPX] = xr.astype(ml_dtypes.bfloat16)
    x_flat = np.empty((B, 128, XOFF[-1]), dtype=ml_dtypes.bfloat16)
    for n, (c0, c1) in enumerate(XBLOCKS):
        x_flat[:, :, XOFF[n]:XOFF[n + 1]] = (
            x_all[:, :, :, c0:c1].reshape(B, 128, KC * (c1 - c0)))

    # w: (64, 512, 3, 3) -> [128, KC, 3(i), 192(j*64+o)] bf16
    wr = weight.reshape(COUT, KC, 128, 3, 3)
    w_dev = wr.transpose(2, 1, 3, 4, 0).reshape(128, KC, 3, 192)
    w_dev = np.ascontiguousarray(w_dev).astype(ml_dtypes.bfloat16)

    # gates, consumed at y-alignment q' (pre-shifted by 1-j):
    #   g_ij[q'] = exp(-a*|d[q] - d[q + off_ij]|), q = q' + 1 - j,
    #   off_ij = 64*(i-1) + (j-1); invalid taps -> exactly 0
    d = depth.reshape(B, H, W).astype(np.float32)
    dflat = d.reshape(B, NPX)
    g_all = np.zeros((B, 128, NT, 9), dtype=np.float32)
    qp = np.arange(NPX)
    for i in range(3):
        for j in range(3):
            q = qp + 1 - j
            q_ok = (q >= 0) & (q < NPX)
            qc = np.clip(q, 0, NPX - 1)
            h_q, w_q = qc // W, qc % W
            hn, wn = h_q + i - 1, w_q + j - 1
            n_ok = q_ok & (hn >= 0) & (hn < H) & (wn >= 0) & (wn < W)
            hnc = np.clip(hn, 0, H - 1)
            wnc = np.clip(wn, 0, W - 1)
            a = dflat[:, qc]                      # d at out pixel
            bV = d[:, hnc, wnc]                   # d at neighbor
            gv = np.exp(-ALPHA * np.abs(a - bV)) * n_ok[None, :]
            # [B, NPX] -> [B, p=(q'%128), t=(q'//128)] ; q' = h*64+w
            g_all[:, :, :, 3 * i + j] = (
                gv.reshape(B, 32, 128).transpose(0, 2, 1))
    # tiles 0-7: 32-wide gates (both cout halves share them);
    # tiles 8-31: full 64-wide gates for the single 2x-mode multiply
    g_rep = np.ascontiguousarray(
        np.broadcast_to(g_all[:, :, :8, :, None], (B, 128, 8, 9, GW))
    ).astype(ml_dtypes.bfloat16)
    g_rep64 = np.ascontiguousarray(
        np.broadcast_to(g_all[:, :, 8:, :, None], (B, 128, NT - 8, 9, COUT))
    ).astype(ml_dtypes.bfloat16)

    return [
        {"x": x_flat[b], "w": w_dev, "g": g_rep[b], "g64": g_rep64[b]}
        for b in range(B)
    ]


def kernel(input, depth, weight):
    input = np.asarray(input, dtype=np.float32)
    depth = np.asarray(depth, dtype=np.float32)
    weight = np.asarray(weight, dtype=np.float32)

    if "nc" not in _cache:
        _cache["nc"] = build_nc()
    nc = _cache["nc"]

    in_maps = prep_inputs(input, depth, weight)
    kwargs = {}
    if os.environ.get("KERNEL_TRACE") == "1":
        kwargs = dict(trace=True, trace_cores=list(range(B)))
    res = run_bass_kernel_spmd(nc, in_maps, core_ids=list(range(B)), **kwargs)
    _cache["last_results"] = res
    # combine the three j-aligned streams: out[q] = P0[q-1] + P1[q] + P2[q+1]
    outs = []
    for b in range(B):
        p3 = res.results[b]["out"].astype(np.float32)   # [NPX, 3, COUT]
        o = p3[:, 1, :].copy()
        o[1:] += p3[:-1, 0, :]
        o[:-1] += p3[1:, 2, :]
        outs.append(o.T.reshape(COUT, H, W))
    return np.stack(outs).astype(np.float32)


if __name__ == "__main__":
    rng = np.random.default_rng(0)
    x = rng.standard_normal((B, CIN, H, W), dtype=np.float32)
    d = rng.random((B, 1, H, W), dtype=np.float32)
    w = (rng.random((COUT, CIN, 3, 3), dtype=np.float32) - 0.5) * 0.08
    o = kernel(x, d, w)
    print(o.shape, o.dtype)
